# revision 11
# baseline (speedup 1.0000x reference)
"""Trainium2 Bass kernel for gated sparse attention (nn_Attention_40664750358615).

Strategy (8 NeuronCores, tensor-parallel over heads):
  - Each core owns 4 of the 32 heads: wq/wk/wv output-dim shard [4096, 512],
    gate shard, adapter K/V shard.
  - Per core: Q/K/V projections (bf16, f32 accumulation), RoPE, causal
    flash-style attention without max-subtraction (scores ~N(0,1)), plus the
    10 adapter keys with a separate softmax scaled by tanh(gate).
  - Scores are computed transposed (S^T[k, q]) so exp() output feeds the PV
    matmul directly as lhsT with no on-chip transpose of P. Row sums ride in
    an extra ones-column appended to V.
  - Output projection: attention outputs are written (transposed, bf16) into
    a DRAM buffer laid out for an AllToAll; after the collective core j holds
    the full 4096-dim attention output for global tokens [j*512, (j+1)*512),
    multiplies by the full wo, and writes that token slice. The host just
    concatenates the 8 slices.

RoPE trick: wq/wk columns are pre-permuted on the host so that each head's
dims are stored [even pairs | odd pairs]; rotation then needs only
consecutive-partition-range ops on the vector engine. The permutation cancels
in q.k dot products (applied to both q and k, and to adapter k).
"""

import numpy as np
import ml_dtypes

BF16 = ml_dtypes.bfloat16

B, S, D = 2, 2048, 4096
H, HD = 32, 128
NCORE = 8
HLOC = H // NCORE          # 4 heads per core
DLOC = HLOC * HD           # 512
TOK = B * S                # 4096 global tokens
TLOC = TOK // NCORE        # 512 tokens per core output slice
ALEN = 10
ALEN_PAD = 16
SCALE = 1.0 / float(np.sqrt(HD))
TT = 256                   # token tile == q-block size
NTT = S // TT              # 8 per batch
NBAND = TT // 128          # diagonal-band k-tiles per q-block (2)
NKT = S // 128             # 16 k-tiles of 128 per batch
NDC = D // 128             # 32 contraction chunks

_BUILT = {}


class _Ctx:
    """Carries the Bass handle, pools, dram handles, and constant tiles."""
    pass


def _load_consts(g):
    nc = g.nc
    g.cosT_sb = g.consts.tile([128, S], g.bf16, tag="cosT")
    g.sinT_sb = g.consts.tile([128, S], g.bf16, tag="sinT")
    nc.gpsimd.dma_start(g.cosT_sb[:], g.cosT[:, :])
    nc.gpsimd.dma_start(g.sinT_sb[:], g.sinT[:, :])
    g.masks_sb = g.consts.tile([128, NBAND, TT], g.bf16, tag="masks")
    nc.gpsimd.dma_start(g.masks_sb[:], g.masks[:, :, :].rearrange("m k q -> k m q"))
    g.ident_sb = g.consts.tile([128, 128], g.bf16, tag="ident")
    nc.gpsimd.dma_start(g.ident_sb[:], g.ident[:, :])
    g.tanhg_sb = g.consts.tile([128, HLOC], g.f32, tag="tanhg")
    nc.gpsimd.dma_start(g.tanhg_sb[:], g.tanhg[:, :])
    g.adpT_sb = g.consts.tile([128, NDC, ALEN_PAD], g.bf16, tag="adpT")
    nc.gpsimd.dma_start(g.adpT_sb[:], g.adpT[:, :].rearrange("(o p) l -> p o l", p=128))


def _load_weights_hp(g, hp):
    nc = g.nc
    g.wq_t = g.wpool.tile([128, NDC, 256], g.bf16, tag="wq")
    g.wk_t = g.wpool.tile([128, NDC, 256], g.bf16, tag="wk")
    g.wv_t = g.wpool.tile([128, NDC, 256], g.bf16, tag="wv")
    csl = slice(hp * 256, (hp + 1) * 256)
    nc.gpsimd.dma_start(g.wq_t[:], g.wq[:, csl].rearrange("(o p) d -> p o d", p=128))
    nc.gpsimd.dma_start(g.wk_t[:], g.wk[:, csl].rearrange("(o p) d -> p o d", p=128))
    nc.gpsimd.dma_start(g.wv_t[:], g.wv[:, csl].rearrange("(o p) d -> p o d", p=128))


def _adapter_kv(g):
    """akT[d, l] per head and av[l, d]+ones for the current head pair."""
    nc = g.nc
    g.akT_t = g.small.tile([128, 2, ALEN_PAD], g.bf16, tag="akT")
    for hl in range(2):
        ps = g.ps_mm.tile([128, 512], g.f32, tag="mm")
        for dc in range(NDC):
            nc.tensor.matmul(
                ps[:, :ALEN_PAD],
                g.wk_t[:, dc, hl * 128 : (hl + 1) * 128],
                g.adpT_sb[:, dc, :],
                start=(dc == 0),
                stop=(dc == NDC - 1),
            )
        nc.scalar.copy(g.akT_t[:, hl, :], ps[:, :ALEN_PAD])
    g.av_t = g.small.tile([ALEN_PAD, 2, 132], g.bf16, tag="av")
    nc.vector.memset(g.av_t[:], 0.0)
    psv = g.ps_mm.tile([128, 512], g.f32, tag="mm")
    for dc in range(NDC):
        nc.tensor.matmul(
            psv[:ALEN_PAD, :256],
            g.adpT_sb[:, dc, :],
            g.wv_t[:, dc, :],
            start=(dc == 0),
            stop=(dc == NDC - 1),
        )
    nc.scalar.copy(
        g.av_t[:, :, 0:128], psv[:ALEN_PAD, :256].rearrange("l (h d) -> l h d", h=2)
    )
    nc.vector.memset(g.av_t[0:ALEN, :, 128:129], 1.0)


def _rope(g, psp, dst, tsl):
    """De-interleaved RoPE: psum [128, TT] -> dst bf16 [128, TT]."""
    nc, Alu = g.nc, g.Alu
    tmp = g.ropep.tile([128, TT], g.bf16, tag="ropetmp")
    nc.scalar.copy(tmp[:], psp[:])
    cs = g.cosT_sb[:, tsl]
    sn = g.sinT_sb[:, tsl]
    m1 = g.ropep.tile([64, TT], g.bf16, tag="m1")
    m2 = g.ropep.tile([64, TT], g.bf16, tag="m2")
    nc.vector.tensor_tensor(m1[:], tmp[0:64, :], cs[0:64, :], op=Alu.mult)
    nc.vector.tensor_tensor(m2[:], tmp[64:128, :], sn[64:128, :], op=Alu.mult)
    nc.vector.tensor_tensor(dst[0:64, :], m1[:], m2[:], op=Alu.subtract)
    m3 = g.ropep.tile([64, TT], g.bf16, tag="m3")
    m4 = g.ropep.tile([64, TT], g.bf16, tag="m4")
    nc.vector.tensor_tensor(m3[:], tmp[0:64, :], sn[0:64, :], op=Alu.mult)
    nc.vector.tensor_tensor(m4[:], tmp[64:128, :], cs[64:128, :], op=Alu.mult)
    nc.vector.tensor_tensor(dst[64:128, :], m3[:], m4[:], op=Alu.add)


def _project_tile(g, b, tt, KT_t, V_t):
    """Load xT tile (DMA transpose), project Q (returned), K, V for this tile."""
    nc = g.nc
    tsl = slice(tt * TT, (tt + 1) * TT)
    xT = g.xtp.tile([128, NDC, TT], g.bf16, tag="xT")
    for dc in range(NDC):
        nc.sync.dma_start_transpose(
            xT[:, dc, :], g.x[b, tsl, dc * 128 : (dc + 1) * 128]
        )
    QT_t = g.qtp.tile([128, 2, TT], g.bf16, tag="QT")
    for hl in range(2):
        wsl = slice(hl * 128, (hl + 1) * 128)
        for w_t, dst in ((g.wq_t, QT_t[:, hl, :]), (g.wk_t, KT_t[:, hl, tsl])):
            psp = g.ps_mm.tile([128, 512], g.f32, tag="mm")
            for dc in range(NDC):
                nc.tensor.matmul(
                    psp[:, :TT], w_t[:, dc, wsl], xT[:, dc, :],
                    start=(dc == 0), stop=(dc == NDC - 1),
                )
            _rope(g, psp[:, :TT], dst, tsl)
    for vs in range(TT // 128):
        psq = g.ps_mm.tile([128, 512], g.f32, tag="mm")
        for dc in range(NDC):
            nc.tensor.matmul(
                psq[:, :256], xT[:, dc, vs * 128 : (vs + 1) * 128], g.wv_t[:, dc, :],
                start=(dc == 0), stop=(dc == NDC - 1),
            )
        kt = tt * (TT // 128) + vs
        nc.scalar.copy(
            V_t[:, kt, :, 0:128], psq[:, :256].rearrange("p (h d) -> p h d", h=2)
        )
        nc.vector.memset(V_t[:, kt, :, 128:129], 1.0)
    return QT_t


def _attend_block(g, b, hp, hl, qt, QT_t, KT_t, V_t, a2a_in):
    """Causal attention for q-block qt (TT wide), one head.

    Output pieces are normalized, transposed, and DMA'd straight into the
    AllToAll DRAM input buffer."""
    nc, Alu, Act = g.nc, g.Alu, g.Act
    nkt = NBAND * qt + NBAND
    ET = g.etp.tile([128, NKT, TT], g.bf16, tag="ET")
    for kt in range(nkt):
        pss = g.ps_sc.tile([128, 512], g.f32, tag="sc")
        nc.tensor.matmul(
            pss[:, :TT], KT_t[:, hl, kt * 128 : (kt + 1) * 128], QT_t[:, hl, :],
            start=True, stop=True,
        )
        nc.scalar.activation(ET[:, kt, :], pss[:, :TT], Act.Exp, scale=SCALE)
        if kt >= NBAND * qt:  # diagonal band: causal 0/1 mask
            nc.vector.tensor_tensor(
                ET[:, kt, :], ET[:, kt, :],
                g.masks_sb[:, kt - NBAND * qt, :], op=Alu.mult,
            )
    psa = g.ps_sc.tile([128, 512], g.f32, tag="sc")
    nc.tensor.matmul(
        psa[:ALEN_PAD, :TT], g.akT_t[:, hl, :], QT_t[:, hl, :], start=True, stop=True
    )
    EaT = g.small.tile([ALEN_PAD, TT], g.bf16, tag="EaT")
    nc.scalar.activation(EaT[:], psa[:ALEN_PAD, :TT], Act.Exp, scale=SCALE)
    h = 2 * hp + hl
    for qs in range(TT // 128):
        q0 = qs * 128
        On = g.ps_o.tile([128, 512], g.f32, tag="o")
        Oa = g.ps_o.tile([128, 512], g.f32, tag="o")
        nk2 = NBAND * qt + qs + 1
        for kt in range(nk2):
            nc.tensor.matmul(
                On[:, :129], ET[:, kt, q0 : q0 + 128], V_t[:, kt, hl, :129],
                start=(kt == 0), stop=(kt == nk2 - 1),
            )
        nc.tensor.matmul(
            Oa[:, :129], EaT[:, q0 : q0 + 128], g.av_t[:, hl, :129],
            start=True, stop=True,
        )
        rn = g.small.tile([128, 1], g.f32, tag="rn")
        ra = g.small.tile([128, 1], g.f32, tag="ra")
        nc.vector.reciprocal(rn[:], On[:, 128:129])
        nc.vector.reciprocal(ra[:], Oa[:, 128:129])
        nc.vector.tensor_tensor(ra[:], ra[:], g.tanhg_sb[:, h : h + 1], op=Alu.mult)
        t1 = g.small.tile([128, 128], g.f32, tag="t1")
        t2 = g.small.tile([128, 128], g.f32, tag="t2")
        nc.vector.tensor_scalar(t1[:], On[:, :128], rn[:], None, op0=Alu.mult)
        nc.vector.tensor_scalar(t2[:], Oa[:, :128], ra[:], None, op0=Alu.mult)
        osb = g.small.tile([128, 128], g.bf16, tag="osb")
        nc.vector.tensor_tensor(osb[:], t1[:], t2[:], op=Alu.add)
        pt = g.ps_t.tile([128, 128], g.bf16, tag="otr")
        nc.tensor.transpose(pt[:], osb[:], g.ident_sb[:])
        osbT = g.small.tile([128, 128], g.bf16, tag="osbT")
        nc.scalar.copy(osbT[:], pt[:])
        tok0 = b * S + qt * TT + q0
        j, c0 = tok0 // TLOC, tok0 % TLOC
        nc.gpsimd.dma_start(
            a2a_in[j * DLOC + h * 128 : j * DLOC + (h + 1) * 128, c0 : c0 + 128],
            osbT[:],
        )


def _wo_stage(g, a2a_out):
    """y[tloc, :] = OTF.T @ wo for this core's token slice."""
    nc = g.nc
    OTF = g.otfp.tile([128, NDC, TLOC], g.bf16, tag="OTF")
    nc.sync.dma_start(OTF[:], a2a_out[:, :].rearrange("(o p) t -> p o t", p=128))
    for mt in range(16):
        wo_t = g.wopool.tile([128, NDC, 256], g.bf16, tag="wo")
        nc.gpsimd.dma_start(
            wo_t[:],
            g.wo[:, mt * 256 : (mt + 1) * 256].rearrange("(o p) m -> p o m", p=128),
        )
        for ts4 in range(TLOC // 128):
            py = g.ps_mm.tile([128, 512], g.f32, tag="mm")
            for dc in range(NDC):
                nc.tensor.matmul(
                    py[:, :256], OTF[:, dc, ts4 * 128 : (ts4 + 1) * 128],
                    wo_t[:, dc, :],
                    start=(dc == 0), stop=(dc == NDC - 1),
                )
            y_sb = g.ysbp.tile([128, 256], g.f32, tag="y")
            nc.scalar.copy(y_sb[:], py[:, :256])
            nc.sync.dma_start(
                g.out[ts4 * 128 : (ts4 + 1) * 128, mt * 256 : (mt + 1) * 256], y_sb[:]
            )


def _body(g, tc):
    from contextlib import ExitStack

    nc, mybir = g.nc, g.mybir
    _load_consts(g)
    a2a_in = g.dram.tile([NCORE * DLOC, TLOC], g.bf16, tag="a2a_in")
    a2a_out = g.dram.tile([NCORE * DLOC, TLOC], g.bf16, tag="a2a_out")

    with ExitStack() as st:
        g.wpool = st.enter_context(tc.tile_pool(name="wpool", bufs=1))
        g.xtp = st.enter_context(tc.tile_pool(name="xt", bufs=2))
        g.qtp = st.enter_context(tc.tile_pool(name="qt", bufs=2))
        g.kvp = st.enter_context(tc.tile_pool(name="kv", bufs=1))
        g.etp = st.enter_context(tc.tile_pool(name="et", bufs=2))
        g.ropep = st.enter_context(tc.tile_pool(name="ropep", bufs=3))
        for hp in range(2):
            _load_weights_hp(g, hp)
            _adapter_kv(g)
            for b in range(B):
                KT_t = g.kvp.tile([128, 2, S], g.bf16, tag="KT")
                V_t = g.kvp.tile([128, NKT, 2, 132], g.bf16, tag="V")
                for tt in range(NTT):
                    QT_t = _project_tile(g, b, tt, KT_t, V_t)
                    for hl in range(2):
                        _attend_block(g, b, hp, hl, tt, QT_t, KT_t, V_t, a2a_in)

    nc.gpsimd.collective_compute(
        "AllToAll",
        mybir.AluOpType.bypass,
        replica_groups=[list(range(NCORE))],
        ins=[a2a_in[:, :].opt()],
        outs=[a2a_out[:, :].opt()],
    )

    with ExitStack() as st:
        g.otfp = st.enter_context(tc.tile_pool(name="otf", bufs=1))
        g.wopool = st.enter_context(tc.tile_pool(name="wopool", bufs=2))
        g.ysbp = st.enter_context(tc.tile_pool(name="ysb", bufs=3))
        _wo_stage(g, a2a_out)


def _build():
    """Build the SPMD Bass graph (same program on all 8 cores)."""
    import concourse.bass as bass  # noqa: F401
    import concourse.mybir as mybir
    import concourse.tile as tile
    from concourse import bacc
    from contextlib import ExitStack

    g = _Ctx()
    g.mybir = mybir
    dt = mybir.dt
    g.f32, g.bf16 = dt.float32, dt.bfloat16
    g.Alu = mybir.AluOpType
    g.Act = mybir.ActivationFunctionType

    nc = bacc.Bacc(num_devices=NCORE, target_bir_lowering=False, debug=False)
    g.nc = nc

    g.x = nc.dram_tensor("x", [B, S, D], g.bf16, kind="ExternalInput")
    g.wq = nc.dram_tensor("wq", [D, DLOC], g.bf16, kind="ExternalInput")
    g.wk = nc.dram_tensor("wk", [D, DLOC], g.bf16, kind="ExternalInput")
    g.wv = nc.dram_tensor("wv", [D, DLOC], g.bf16, kind="ExternalInput")
    g.wo = nc.dram_tensor("wo", [D, D], g.bf16, kind="ExternalInput")
    g.adpT = nc.dram_tensor("adpT", [D, ALEN_PAD], g.bf16, kind="ExternalInput")
    g.cosT = nc.dram_tensor("cosT", [128, S], g.bf16, kind="ExternalInput")
    g.sinT = nc.dram_tensor("sinT", [128, S], g.bf16, kind="ExternalInput")
    g.tanhg = nc.dram_tensor("tanhg", [128, HLOC], g.f32, kind="ExternalInput")
    g.masks = nc.dram_tensor("masks", [NBAND, 128, TT], g.bf16, kind="ExternalInput")
    g.ident = nc.dram_tensor("ident", [128, 128], g.bf16, kind="ExternalInput")
    g.out = nc.dram_tensor("out", [TLOC, D], g.f32, kind="ExternalOutput")

    with ExitStack() as st:
        tc = st.enter_context(tile.TileContext(nc))
        g.consts = st.enter_context(tc.tile_pool(name="consts", bufs=1))
        g.small = st.enter_context(tc.tile_pool(name="small", bufs=3))
        g.ps_mm = st.enter_context(tc.tile_pool(name="ps_mm", bufs=2, space="PSUM"))
        g.ps_sc = st.enter_context(tc.tile_pool(name="ps_sc", bufs=2, space="PSUM"))
        g.ps_o = st.enter_context(tc.tile_pool(name="ps_o", bufs=2, space="PSUM"))
        g.ps_t = st.enter_context(tc.tile_pool(name="ps_t", bufs=2, space="PSUM"))
        g.dram = st.enter_context(tc.tile_pool(name="dram", bufs=1, space="DRAM"))
        _body(g, tc)

    nc.finalize()
    return nc


def _host_inputs(x, wq, wk, wv, wo, gate, adapter, freqs_cos, freqs_sin):
    """Host-side preprocessing: dtype casts, head sharding, RoPE de-interleave
    permutation of wq/wk columns, small derived tensors."""
    x = np.asarray(x, np.float32)
    wq = np.asarray(wq, np.float32)
    wk = np.asarray(wk, np.float32)
    wv = np.asarray(wv, np.float32)
    wo = np.asarray(wo, np.float32)
    gate = np.asarray(gate, np.float32).reshape(H)
    adapter = np.asarray(adapter, np.float32).reshape(ALEN, D)
    cos = np.asarray(freqs_cos, np.float32)
    sin = np.asarray(freqs_sin, np.float32)

    # de-interleave permutation within each head's 128 output dims
    perm = np.concatenate([np.arange(0, HD, 2), np.arange(1, HD, 2)])
    full_perm = (np.arange(H)[:, None] * HD + perm[None, :]).reshape(-1)
    wq_p = wq[:, full_perm]
    wk_p = wk[:, full_perm]

    x_bf = x.astype(BF16)
    wo_bf = wo.astype(BF16)
    adpT = np.zeros((D, ALEN_PAD), np.float32)
    adpT[:, :ALEN] = adapter.T
    adpT_bf = adpT.astype(BF16)
    # rows 0-63 and 64-127 hold the same values (partition-matched operands)
    cosT_bf = np.tile(np.ascontiguousarray(cos.T), (2, 1)).astype(BF16)
    sinT_bf = np.tile(np.ascontiguousarray(sin.T), (2, 1)).astype(BF16)
    ident = np.eye(128, dtype=np.float32).astype(BF16)

    # diagonal-band causal masks: mask[d][k, q] = 1 if k + d*128 <= q
    kk = np.arange(128)[:, None]
    qq = np.arange(TT)[None, :]
    masks = np.stack(
        [(kk + d * 128 <= qq).astype(np.float32) for d in range(NBAND)]
    ).astype(BF16)

    in_maps = []
    for c in range(NCORE):
        hsl = slice(c * DLOC, (c + 1) * DLOC)
        tg = np.tanh(gate[c * HLOC : (c + 1) * HLOC]).astype(np.float32)
        in_maps.append(
            {
                "x": x_bf,
                "wq": np.ascontiguousarray(wq_p[:, hsl]).astype(BF16),
                "wk": np.ascontiguousarray(wk_p[:, hsl]).astype(BF16),
                "wv": np.ascontiguousarray(wv[:, hsl]).astype(BF16),
                "wo": wo_bf,
                "adpT": adpT_bf,
                "cosT": cosT_bf,
                "sinT": sinT_bf,
                "tanhg": np.broadcast_to(tg, (128, HLOC)).copy(),
                "masks": masks,
                "ident": ident,
            }
        )
    return in_maps


def _numpy_fallback(x, wq, wk, wv, wo, gate, adapter, freqs_cos, freqs_sin, mask):
    """Reference-equivalent numpy path (used only if random_init != 0)."""
    x = np.asarray(x, np.float32)
    wq = np.asarray(wq, np.float32)
    wk = np.asarray(wk, np.float32)
    wv = np.asarray(wv, np.float32)
    wo = np.asarray(wo, np.float32)
    cos = np.asarray(freqs_cos, np.float32)
    sin = np.asarray(freqs_sin, np.float32)
    b, s, d = x.shape
    xq = (x @ wq).reshape(b, s, H, HD)
    xk = (x @ wk).reshape(b, s, H, HD)
    xv = (x @ wv).reshape(b, s, H, HD)

    def rope(t):
        t2 = t.reshape(b, s, H, HD // 2, 2)
        tr, ti = t2[..., 0], t2[..., 1]
        c = cos[None, :, None, :]
        sn = sin[None, :, None, :]
        return np.stack([tr * c - ti * sn, tr * sn + ti * c], -1).reshape(b, s, H, HD)

    xq, xk = rope(xq), rope(xk)
    ad = np.asarray(adapter, np.float32).reshape(ALEN, d)
    ak = (ad @ wk).reshape(ALEN, H, HD)
    av = (ad @ wv).reshape(ALEN, H, HD)
    sc_n = np.einsum("bqhd,bkhd->bhqk", xq, xk) / np.sqrt(HD) + np.asarray(
        mask, np.float32
    )
    sc_a = np.einsum("bqhd,lhd->bhql", xq, ak) / np.sqrt(HD)
    cat = np.concatenate([sc_a, sc_n], -1)
    cat = cat - cat.max(-1, keepdims=True)
    e = np.exp(cat)
    p = e / e.sum(-1, keepdims=True)
    pa, pn = p[..., :ALEN], p[..., ALEN:]
    o = np.einsum("bhql,lhd->bqhd", pa, av) + np.einsum("bhqk,bkhd->bqhd", pn, xv)
    return (o.reshape(b, s, d) @ wo).astype(np.float32)


def kernel(x, wq, wk, wv, wo, gate, adapter, freqs_cos, freqs_sin, mask,
           start_pos, random_init, **_unused):
    if int(np.asarray(random_init)) != 0:
        return _numpy_fallback(
            x, wq, wk, wv, wo, gate, adapter, freqs_cos, freqs_sin, mask
        )

    from concourse.bass_utils import run_bass_kernel_spmd

    if "nc" not in _BUILT:
        _BUILT["nc"] = _build()
    nc = _BUILT["nc"]

    in_maps = _host_inputs(x, wq, wk, wv, wo, gate, adapter, freqs_cos, freqs_sin)
    res = run_bass_kernel_spmd(nc, in_maps, core_ids=list(range(NCORE)))
    slices = [np.asarray(r["out"], np.float32) for r in res.results]
    return np.concatenate(slices, axis=0).reshape(B, S, D)


# revision 16
# speedup vs baseline: 1.1847x; 1.1847x over previous
"""Trainium2 Bass kernel for gated sparse attention (nn_Attention_40664750358615).

Strategy (8 NeuronCores, tensor-parallel over heads):
  - Each core owns 4 of the 32 heads: wq/wk/wv output-dim shard [4096, 512],
    gate shard, adapter K/V shard.
  - Per core: Q/K/V projections (bf16, f32 accumulation), RoPE, causal
    flash-style attention without max-subtraction (scores ~N(0,1)), plus the
    10 adapter keys with a separate softmax scaled by tanh(gate).
  - Scores are computed transposed (S^T[k, q]) so exp() output feeds the PV
    matmul directly as lhsT with no on-chip transpose of P. Row sums ride in
    an extra ones-column appended to V.
  - Output projection: attention outputs are written (transposed, bf16) into
    a DRAM buffer laid out for an AllToAll; after the collective core j holds
    the full 4096-dim attention output for global tokens [j*512, (j+1)*512),
    multiplies by the full wo, and writes that token slice. The host just
    concatenates the 8 slices.

RoPE trick: wq/wk columns are pre-permuted on the host so that each head's
dims are stored [even pairs | odd pairs]; rotation then needs only
consecutive-partition-range ops on the vector engine. The permutation cancels
in q.k dot products (applied to both q and k, and to adapter k).
"""

import numpy as np
import ml_dtypes

BF16 = ml_dtypes.bfloat16

B, S, D = 2, 2048, 4096
H, HD = 32, 128
NCORE = 8
HLOC = H // NCORE          # 4 heads per core
DLOC = HLOC * HD           # 512
TOK = B * S                # 4096 global tokens
TLOC = TOK // NCORE        # 512 tokens per core output slice
ALEN = 10
ALEN_PAD = 16
SCALE = 1.0 / float(np.sqrt(HD))
TT = 256                   # token tile == q-block size
NTT = S // TT              # 8 per batch
NBAND = TT // 128          # diagonal-band k-tiles per q-block (2)
NKT = S // 128             # 16 k-tiles of 128 per batch
NDC = D // 128             # 32 contraction chunks

_BUILT = {}


class _Ctx:
    """Carries the Bass handle, pools, dram handles, and constant tiles."""
    pass


def _load_consts(g):
    nc = g.nc
    g.cosT_sb = g.consts.tile([128, S], g.bf16, tag="cosT")
    g.sinT_sb = g.consts.tile([128, S], g.bf16, tag="sinT")
    nc.gpsimd.dma_start(g.cosT_sb[:], g.cosT[:, :])
    nc.gpsimd.dma_start(g.sinT_sb[:], g.sinT[:, :])
    g.masks_sb = g.consts.tile([128, NBAND, TT], g.bf16, tag="masks")
    nc.gpsimd.dma_start(g.masks_sb[:], g.masks[:, :, :].rearrange("m k q -> k m q"))
    g.ident_sb = g.consts.tile([128, 128], g.bf16, tag="ident")
    nc.gpsimd.dma_start(g.ident_sb[:], g.ident[:, :])
    g.tanhg_sb = g.consts.tile([128, HLOC], g.f32, tag="tanhg")
    nc.gpsimd.dma_start(g.tanhg_sb[:], g.tanhg[:, :])
    g.adpT_sb = g.consts.tile([128, NDC, ALEN_PAD], g.bf16, tag="adpT")
    nc.gpsimd.dma_start(g.adpT_sb[:], g.adpT[:, :].rearrange("(o p) l -> p o l", p=128))


def _load_weights_hp(g, hp):
    nc = g.nc
    g.wq_t = g.wpool.tile([128, NDC, 256], g.bf16, tag="wq")
    g.wk_t = g.wpool.tile([128, NDC, 256], g.bf16, tag="wk")
    g.wv_t = g.wpool.tile([128, NDC, 256], g.bf16, tag="wv")
    csl = slice(hp * 256, (hp + 1) * 256)
    nc.gpsimd.dma_start(g.wq_t[:], g.wq[:, csl].rearrange("(o p) d -> p o d", p=128))
    nc.gpsimd.dma_start(g.wk_t[:], g.wk[:, csl].rearrange("(o p) d -> p o d", p=128))
    nc.gpsimd.dma_start(g.wv_t[:], g.wv[:, csl].rearrange("(o p) d -> p o d", p=128))


def _adapter_kv(g):
    """akT[d, l] per head and av[l, d]+ones for the current head pair."""
    nc = g.nc
    g.akT_t = g.small.tile([128, 2, ALEN_PAD], g.bf16, tag="akT")
    for hl in range(2):
        ps = g.ps_mm.tile([128, 512], g.f32, tag="mm")
        for dc in range(NDC):
            nc.tensor.matmul(
                ps[:, :ALEN_PAD],
                g.wk_t[:, dc, hl * 128 : (hl + 1) * 128],
                g.adpT_sb[:, dc, :],
                start=(dc == 0),
                stop=(dc == NDC - 1),
            )
        nc.scalar.copy(g.akT_t[:, hl, :], ps[:, :ALEN_PAD])
    g.av_t = g.small.tile([ALEN_PAD, 2, 132], g.bf16, tag="av")
    nc.vector.memset(g.av_t[:], 0.0)
    psv = g.ps_mm.tile([128, 512], g.f32, tag="mm")
    for dc in range(NDC):
        nc.tensor.matmul(
            psv[:ALEN_PAD, :256],
            g.adpT_sb[:, dc, :],
            g.wv_t[:, dc, :],
            start=(dc == 0),
            stop=(dc == NDC - 1),
        )
    nc.scalar.copy(
        g.av_t[:, :, 0:128], psv[:ALEN_PAD, :256].rearrange("l (h d) -> l h d", h=2)
    )
    nc.vector.memset(g.av_t[0:ALEN, :, 128:129], 1.0)


def _rope(g, psp, dst, tsl):
    """De-interleaved RoPE: psum [128, TT] -> dst bf16 [128, TT]."""
    nc, Alu = g.nc, g.Alu
    tmp = g.ropep.tile([128, TT], g.bf16, tag="ropetmp")
    nc.scalar.copy(tmp[:], psp[:])
    cs = g.cosT_sb[:, tsl]
    sn = g.sinT_sb[:, tsl]
    m1 = g.ropep.tile([64, TT], g.bf16, tag="m1")
    m2 = g.ropep.tile([64, TT], g.bf16, tag="m2")
    nc.vector.tensor_tensor(m1[:], tmp[0:64, :], cs[0:64, :], op=Alu.mult)
    nc.vector.tensor_tensor(m2[:], tmp[64:128, :], sn[64:128, :], op=Alu.mult)
    nc.vector.tensor_tensor(dst[0:64, :], m1[:], m2[:], op=Alu.subtract)
    m3 = g.ropep.tile([64, TT], g.bf16, tag="m3")
    m4 = g.ropep.tile([64, TT], g.bf16, tag="m4")
    nc.vector.tensor_tensor(m3[:], tmp[0:64, :], sn[0:64, :], op=Alu.mult)
    nc.vector.tensor_tensor(m4[:], tmp[64:128, :], cs[64:128, :], op=Alu.mult)
    nc.vector.tensor_tensor(dst[64:128, :], m3[:], m4[:], op=Alu.add)


def _project_tile(g, b, tt, KT_t, V_t, xT_cache, first_pass):
    """Load xT tile (DMA transpose on the first pass, cached in DRAM for the
    second), project Q (returned), K, V for this tile."""
    nc = g.nc
    tsl = slice(tt * TT, (tt + 1) * TT)
    xT = g.xtp.tile([128, NDC, TT], g.bf16, tag="xT")
    if first_pass:
        for dc in range(NDC):
            nc.sync.dma_start_transpose(
                xT[:, dc, :], g.x[b, tsl, dc * 128 : (dc + 1) * 128]
            )
        nc.gpsimd.dma_start(xT_cache[b * NTT + tt], xT[:])
    else:
        nc.gpsimd.dma_start(xT[:], xT_cache[b * NTT + tt])
    QT_t = g.qtp.tile([128, 2, TT], g.bf16, tag="QT")
    for hl in range(2):
        wsl = slice(hl * 128, (hl + 1) * 128)
        for w_t, dst in ((g.wq_t, QT_t[:, hl, :]), (g.wk_t, KT_t[:, hl, tsl])):
            psp = g.ps_mm.tile([128, 512], g.f32, tag="mm")
            for dc in range(NDC):
                nc.tensor.matmul(
                    psp[:, :TT], w_t[:, dc, wsl], xT[:, dc, :],
                    start=(dc == 0), stop=(dc == NDC - 1),
                )
            _rope(g, psp[:, :TT], dst, tsl)
    for vs in range(TT // 128):
        psq = g.ps_mm.tile([128, 512], g.f32, tag="mm")
        for dc in range(NDC):
            nc.tensor.matmul(
                psq[:, :256], xT[:, dc, vs * 128 : (vs + 1) * 128], g.wv_t[:, dc, :],
                start=(dc == 0), stop=(dc == NDC - 1),
            )
        kt = tt * (TT // 128) + vs
        nc.vector.tensor_copy(
            V_t[:, kt, :, 0:128], psq[:, :256].rearrange("p (h d) -> p h d", h=2)
        )
        nc.vector.memset(V_t[:, kt, :, 128:129], 1.0)
    return QT_t


def _attend_block(g, b, hp, hl, qt, QT_t, KT_t, V_t, a2a_in):
    """Causal attention for q-block qt (TT wide), one head.

    Output pieces are normalized, transposed, and DMA'd straight into the
    AllToAll DRAM input buffer."""
    nc, Alu, Act = g.nc, g.Alu, g.Act
    nkt = NBAND * qt + NBAND
    ET = g.etp.tile([128, NKT, TT], g.bf16, tag="ET")
    for kt in range(nkt):
        pss = g.ps_sc.tile([128, 512], g.f32, tag="sc")
        nc.tensor.matmul(
            pss[:, :TT], KT_t[:, hl, kt * 128 : (kt + 1) * 128], QT_t[:, hl, :],
            start=True, stop=True,
        )
        nc.scalar.activation(ET[:, kt, :], pss[:, :TT], Act.Exp, scale=SCALE)
        if kt >= NBAND * qt:  # diagonal band: causal 0/1 mask
            nc.vector.tensor_tensor(
                ET[:, kt, :], ET[:, kt, :],
                g.masks_sb[:, kt - NBAND * qt, :], op=Alu.mult,
            )
    psa = g.ps_sc.tile([128, 512], g.f32, tag="sc")
    nc.tensor.matmul(
        psa[:ALEN_PAD, :TT], g.akT_t[:, hl, :], QT_t[:, hl, :], start=True, stop=True
    )
    EaT = g.small.tile([ALEN_PAD, TT], g.bf16, tag="EaT")
    nc.scalar.activation(EaT[:], psa[:ALEN_PAD, :TT], Act.Exp, scale=SCALE)
    h = 2 * hp + hl
    for qs in range(TT // 128):
        q0 = qs * 128
        On = g.ps_o.tile([128, 512], g.f32, tag="o")
        Oa = g.ps_o.tile([128, 512], g.f32, tag="o")
        nk2 = NBAND * qt + qs + 1
        for kt in range(nk2):
            nc.tensor.matmul(
                On[:, :129], ET[:, kt, q0 : q0 + 128], V_t[:, kt, hl, :129],
                start=(kt == 0), stop=(kt == nk2 - 1),
            )
        nc.tensor.matmul(
            Oa[:, :129], EaT[:, q0 : q0 + 128], g.av_t[:, hl, :129],
            start=True, stop=True,
        )
        rn = g.small.tile([128, 1], g.f32, tag="rn")
        ra = g.small.tile([128, 1], g.f32, tag="ra")
        nc.vector.reciprocal(rn[:], On[:, 128:129])
        nc.vector.reciprocal(ra[:], Oa[:, 128:129])
        nc.vector.tensor_tensor(ra[:], ra[:], g.tanhg_sb[:, h : h + 1], op=Alu.mult)
        t1 = g.small.tile([128, 128], g.f32, tag="t1")
        t2 = g.small.tile([128, 128], g.f32, tag="t2")
        nc.vector.tensor_scalar(t1[:], On[:, :128], rn[:], None, op0=Alu.mult)
        nc.vector.tensor_scalar(t2[:], Oa[:, :128], ra[:], None, op0=Alu.mult)
        osb = g.small.tile([128, 128], g.bf16, tag="osb")
        nc.vector.tensor_tensor(osb[:], t1[:], t2[:], op=Alu.add)
        pt = g.ps_t.tile([128, 128], g.bf16, tag="otr")
        nc.tensor.transpose(pt[:], osb[:], g.ident_sb[:])
        osbT = g.small.tile([128, 128], g.bf16, tag="osbT")
        nc.vector.tensor_copy(osbT[:], pt[:])
        tok0 = b * S + qt * TT + q0
        j, c0 = tok0 // TLOC, tok0 % TLOC
        nc.gpsimd.dma_start(
            a2a_in[j * DLOC + h * 128 : j * DLOC + (h + 1) * 128, c0 : c0 + 128],
            osbT[:],
        )


def _wo_stage(g, a2a_out):
    """y[tloc, :] = OTF.T @ wo for this core's token slice."""
    nc = g.nc
    OTF = g.otfp.tile([128, NDC, TLOC], g.bf16, tag="OTF")
    nc.sync.dma_start(OTF[:], a2a_out[:, :].rearrange("(o p) t -> p o t", p=128))
    for mt in range(16):
        wo_t = g.wopool.tile([128, NDC, 256], g.bf16, tag="wo")
        nc.gpsimd.dma_start(
            wo_t[:],
            g.wo[:, mt * 256 : (mt + 1) * 256].rearrange("(o p) m -> p o m", p=128),
        )
        for ts4 in range(TLOC // 128):
            py = g.ps_mm.tile([128, 512], g.f32, tag="mm")
            for dc in range(NDC):
                nc.tensor.matmul(
                    py[:, :256], OTF[:, dc, ts4 * 128 : (ts4 + 1) * 128],
                    wo_t[:, dc, :],
                    start=(dc == 0), stop=(dc == NDC - 1),
                )
            y_sb = g.ysbp.tile([128, 256], g.f32, tag="y")
            nc.scalar.copy(y_sb[:], py[:, :256])
            nc.sync.dma_start(
                g.out[ts4 * 128 : (ts4 + 1) * 128, mt * 256 : (mt + 1) * 256], y_sb[:]
            )


def _body(g, tc):
    from contextlib import ExitStack

    nc, mybir = g.nc, g.mybir
    _load_consts(g)
    a2a_in = g.dram.tile([NCORE * DLOC, TLOC], g.bf16, tag="a2a_in")
    a2a_out = g.dram.tile([NCORE * DLOC, TLOC], g.bf16, tag="a2a_out")
    xT_cache = g.dram.tile([B * NTT, 128, NDC, TT], g.bf16, tag="xTc")

    with ExitStack() as st:
        g.wpool = st.enter_context(tc.tile_pool(name="wpool", bufs=1))
        g.xtp = st.enter_context(tc.tile_pool(name="xt", bufs=2))
        g.qtp = st.enter_context(tc.tile_pool(name="qt", bufs=2))
        g.kvp = st.enter_context(tc.tile_pool(name="kv", bufs=1))
        g.etp = st.enter_context(tc.tile_pool(name="et", bufs=2))
        g.ropep = st.enter_context(tc.tile_pool(name="ropep", bufs=3))
        for hp in range(2):
            _load_weights_hp(g, hp)
            _adapter_kv(g)
            for b in range(B):
                KT_t = g.kvp.tile([128, 2, S], g.bf16, tag="KT")
                V_t = g.kvp.tile([128, NKT, 2, 132], g.bf16, tag="V")
                for tt in range(NTT):
                    QT_t = _project_tile(g, b, tt, KT_t, V_t, xT_cache, hp == 0)
                    for hl in range(2):
                        _attend_block(g, b, hp, hl, tt, QT_t, KT_t, V_t, a2a_in)

    nc.gpsimd.collective_compute(
        "AllToAll",
        mybir.AluOpType.bypass,
        replica_groups=[list(range(NCORE))],
        ins=[a2a_in[:, :].opt()],
        outs=[a2a_out[:, :].opt()],
    )

    with ExitStack() as st:
        g.otfp = st.enter_context(tc.tile_pool(name="otf", bufs=1))
        g.wopool = st.enter_context(tc.tile_pool(name="wopool", bufs=2))
        g.ysbp = st.enter_context(tc.tile_pool(name="ysb", bufs=3))
        _wo_stage(g, a2a_out)


def _build():
    """Build the SPMD Bass graph (same program on all 8 cores)."""
    import concourse.bass as bass  # noqa: F401
    import concourse.mybir as mybir
    import concourse.tile as tile
    from concourse import bacc
    from contextlib import ExitStack

    g = _Ctx()
    g.mybir = mybir
    dt = mybir.dt
    g.f32, g.bf16 = dt.float32, dt.bfloat16
    g.Alu = mybir.AluOpType
    g.Act = mybir.ActivationFunctionType

    nc = bacc.Bacc(num_devices=NCORE, target_bir_lowering=False, debug=False)
    g.nc = nc

    g.x = nc.dram_tensor("x", [B, S, D], g.bf16, kind="ExternalInput")
    g.wq = nc.dram_tensor("wq", [D, DLOC], g.bf16, kind="ExternalInput")
    g.wk = nc.dram_tensor("wk", [D, DLOC], g.bf16, kind="ExternalInput")
    g.wv = nc.dram_tensor("wv", [D, DLOC], g.bf16, kind="ExternalInput")
    g.wo = nc.dram_tensor("wo", [D, D], g.bf16, kind="ExternalInput")
    g.adpT = nc.dram_tensor("adpT", [D, ALEN_PAD], g.bf16, kind="ExternalInput")
    g.cosT = nc.dram_tensor("cosT", [128, S], g.bf16, kind="ExternalInput")
    g.sinT = nc.dram_tensor("sinT", [128, S], g.bf16, kind="ExternalInput")
    g.tanhg = nc.dram_tensor("tanhg", [128, HLOC], g.f32, kind="ExternalInput")
    g.masks = nc.dram_tensor("masks", [NBAND, 128, TT], g.bf16, kind="ExternalInput")
    g.ident = nc.dram_tensor("ident", [128, 128], g.bf16, kind="ExternalInput")
    g.out = nc.dram_tensor("out", [TLOC, D], g.f32, kind="ExternalOutput")

    with ExitStack() as st:
        tc = st.enter_context(tile.TileContext(nc))
        g.consts = st.enter_context(tc.tile_pool(name="consts", bufs=1))
        g.small = st.enter_context(tc.tile_pool(name="small", bufs=3))
        g.ps_mm = st.enter_context(tc.tile_pool(name="ps_mm", bufs=2, space="PSUM"))
        g.ps_sc = st.enter_context(tc.tile_pool(name="ps_sc", bufs=2, space="PSUM"))
        g.ps_o = st.enter_context(tc.tile_pool(name="ps_o", bufs=2, space="PSUM"))
        g.ps_t = st.enter_context(tc.tile_pool(name="ps_t", bufs=2, space="PSUM"))
        g.dram = st.enter_context(tc.tile_pool(name="dram", bufs=1, space="DRAM"))
        _body(g, tc)

    nc.finalize()
    return nc


def _host_inputs(x, wq, wk, wv, wo, gate, adapter, freqs_cos, freqs_sin):
    """Host-side preprocessing: dtype casts, head sharding, RoPE de-interleave
    permutation of wq/wk columns, small derived tensors."""
    x = np.asarray(x, np.float32)
    wq = np.asarray(wq, np.float32)
    wk = np.asarray(wk, np.float32)
    wv = np.asarray(wv, np.float32)
    wo = np.asarray(wo, np.float32)
    gate = np.asarray(gate, np.float32).reshape(H)
    adapter = np.asarray(adapter, np.float32).reshape(ALEN, D)
    cos = np.asarray(freqs_cos, np.float32)
    sin = np.asarray(freqs_sin, np.float32)

    # de-interleave permutation within each head's 128 output dims
    perm = np.concatenate([np.arange(0, HD, 2), np.arange(1, HD, 2)])
    full_perm = (np.arange(H)[:, None] * HD + perm[None, :]).reshape(-1)
    wq_p = wq[:, full_perm]
    wk_p = wk[:, full_perm]

    x_bf = x.astype(BF16)
    wo_bf = wo.astype(BF16)
    adpT = np.zeros((D, ALEN_PAD), np.float32)
    adpT[:, :ALEN] = adapter.T
    adpT_bf = adpT.astype(BF16)
    # rows 0-63 and 64-127 hold the same values (partition-matched operands)
    cosT_bf = np.tile(np.ascontiguousarray(cos.T), (2, 1)).astype(BF16)
    sinT_bf = np.tile(np.ascontiguousarray(sin.T), (2, 1)).astype(BF16)
    ident = np.eye(128, dtype=np.float32).astype(BF16)

    # diagonal-band causal masks: mask[d][k, q] = 1 if k + d*128 <= q
    kk = np.arange(128)[:, None]
    qq = np.arange(TT)[None, :]
    masks = np.stack(
        [(kk + d * 128 <= qq).astype(np.float32) for d in range(NBAND)]
    ).astype(BF16)

    in_maps = []
    for c in range(NCORE):
        hsl = slice(c * DLOC, (c + 1) * DLOC)
        tg = np.tanh(gate[c * HLOC : (c + 1) * HLOC]).astype(np.float32)
        in_maps.append(
            {
                "x": x_bf,
                "wq": np.ascontiguousarray(wq_p[:, hsl]).astype(BF16),
                "wk": np.ascontiguousarray(wk_p[:, hsl]).astype(BF16),
                "wv": np.ascontiguousarray(wv[:, hsl]).astype(BF16),
                "wo": wo_bf,
                "adpT": adpT_bf,
                "cosT": cosT_bf,
                "sinT": sinT_bf,
                "tanhg": np.broadcast_to(tg, (128, HLOC)).copy(),
                "masks": masks,
                "ident": ident,
            }
        )
    return in_maps


def _numpy_fallback(x, wq, wk, wv, wo, gate, adapter, freqs_cos, freqs_sin, mask):
    """Reference-equivalent numpy path (used only if random_init != 0)."""
    x = np.asarray(x, np.float32)
    wq = np.asarray(wq, np.float32)
    wk = np.asarray(wk, np.float32)
    wv = np.asarray(wv, np.float32)
    wo = np.asarray(wo, np.float32)
    cos = np.asarray(freqs_cos, np.float32)
    sin = np.asarray(freqs_sin, np.float32)
    b, s, d = x.shape
    xq = (x @ wq).reshape(b, s, H, HD)
    xk = (x @ wk).reshape(b, s, H, HD)
    xv = (x @ wv).reshape(b, s, H, HD)

    def rope(t):
        t2 = t.reshape(b, s, H, HD // 2, 2)
        tr, ti = t2[..., 0], t2[..., 1]
        c = cos[None, :, None, :]
        sn = sin[None, :, None, :]
        return np.stack([tr * c - ti * sn, tr * sn + ti * c], -1).reshape(b, s, H, HD)

    xq, xk = rope(xq), rope(xk)
    ad = np.asarray(adapter, np.float32).reshape(ALEN, d)
    ak = (ad @ wk).reshape(ALEN, H, HD)
    av = (ad @ wv).reshape(ALEN, H, HD)
    sc_n = np.einsum("bqhd,bkhd->bhqk", xq, xk) / np.sqrt(HD) + np.asarray(
        mask, np.float32
    )
    sc_a = np.einsum("bqhd,lhd->bhql", xq, ak) / np.sqrt(HD)
    cat = np.concatenate([sc_a, sc_n], -1)
    cat = cat - cat.max(-1, keepdims=True)
    e = np.exp(cat)
    p = e / e.sum(-1, keepdims=True)
    pa, pn = p[..., :ALEN], p[..., ALEN:]
    o = np.einsum("bhql,lhd->bqhd", pa, av) + np.einsum("bhqk,bkhd->bqhd", pn, xv)
    return (o.reshape(b, s, d) @ wo).astype(np.float32)


def kernel(x, wq, wk, wv, wo, gate, adapter, freqs_cos, freqs_sin, mask,
           start_pos, random_init, **_unused):
    if int(np.asarray(random_init)) != 0:
        return _numpy_fallback(
            x, wq, wk, wv, wo, gate, adapter, freqs_cos, freqs_sin, mask
        )

    from concourse.bass_utils import run_bass_kernel_spmd

    if "nc" not in _BUILT:
        _BUILT["nc"] = _build()
    nc = _BUILT["nc"]

    in_maps = _host_inputs(x, wq, wk, wv, wo, gate, adapter, freqs_cos, freqs_sin)
    res = run_bass_kernel_spmd(nc, in_maps, core_ids=list(range(NCORE)))
    slices = [np.asarray(r["out"], np.float32) for r in res.results]
    return np.concatenate(slices, axis=0).reshape(B, S, D)
